# revision 26
# baseline (speedup 1.0000x reference)
"""Trainium2 Bass kernel for nn_EnhancedSeasonalModule (v2, bf16 compute).

Computation (reference):
  cyc[b,s,:]   = cycle_data[(cycle_index[b]+s) % CL]
  combined     = seasonal * cyc                              [B,S,N,C]
  transformed  = combined @ W_c^T + (lin_b + b_c)            (einsum bsnc,dc->bsnd)
  z            = depthwise_conv1d_k3_same(transformed, conv_w)  over s, per (b,n)
  y            = gelu_exact(z + conv_b)
  ln           = layernorm_C(y) * ln_w + ln_b
  out          = seasonal + gamma * ln

Strategy: data-parallel over batch (2 of 16 per core, 8 cores).
Per (b,n) tile [C=128 x S=288], channels on partitions; all bulk tensors are
bf16 (tolerance 2e-2 >> bf16 rounding):
  - x is DMA'd with an fp32->bf16 cast (SWDGE) in 6 large CONTIGUOUS
    per-(b, n-half, s-chunk) transfers (strided cast-DMAs cost ~2k Q7-built
    descriptors each and were ~130us slower); PE transposes bf16 [s,c]
    tiles into bf16 PSUM; DVE evacuates fused with the cyc multiply at
    2x_1P rate into a padded cb buffer whose edge columns stay zero,
    turning depthwise conv + linear into 3 PSUM-accumulated bf16 matmuls
    with A_k = diag(conv_w[:,k]) @ W_c; ACT applies exact GELU (interior
    linear bias folded; per-edge-column biases via 2 tiny ACTs); y^2 runs
    on the idle GPSIMD engine; LN channel sums come from shifted-ones
    matmuls accumulated into per-n-batch PSUM stats banks; the finish
    transposes y back to token layout (bf16 PSUM) and applies
    out = pp[s]*y^T + (x + qq[s]) with pp = gamma*rstd, qq = -mu*pp,
    split between DVE (tensor_scalar + scalar_tensor_tensor) and ACT
    (Identity with per-partition scale/bias + DVE add) to balance engines.
  - output is written bf16 and upcast on the host.
"""

import numpy as np
import ml_dtypes
from contextlib import ExitStack

import concourse.bass as bass
import concourse.bacc as bacc_mod
import concourse.tile as tile
from concourse import mybir
from concourse.bass_utils import run_bass_kernel_spmd
from concourse.masks import make_identity

F32 = mybir.dt.float32
BF16 = mybir.dt.bfloat16
AF = mybir.ActivationFunctionType
OP = mybir.AluOpType

B, S, N, C, CL = 16, 288, 170, 128, 24
LN_EPS = 1e-5
NCORES = 8
TB_MAX = 43


def _chunks(s_total):
    out = []
    s0 = 0
    while s0 < s_total:
        sc = min(128, s_total - s0)
        out.append((s0, sc))
        s0 += sc
    return out


def build_program(
    b_per_core: int,
    n_total: int,
    s_total: int,
    gamma_sc: float,
    tb_max: int = TB_MAX,
    nb: int = 16,
    use_f32r: bool = True,   # kept for signature compat; unused in v2
    fast_path: bool = True,
    act_fn=None,
    repeat: int = 1,
    ablate: str = "",
    pipe_mode: int = 1,
    chunk_modes=("xq", "act", "xq"),  # per-chunk LN-apply strategy
    y2_engine: str = "pool",  # pool | dve | act
    xq_engine: str = "dve",   # dve | pool
    in_mode: str = "swdge_big",  # swdge_big | swdge_bf16 | hwdge_f32
):
    nc = bacc_mod.Bacc("TRN2", target_bir_lowering=False)
    if act_fn is None:
        act_fn = AF.Gelu

    CH = _chunks(s_total)
    SP = s_total + 4  # cb layout: [pad, zero | combined (2..2+S) | zero, pad]

    x_d = nc.declare_dram_parameter("x", [b_per_core, s_total, n_total, C], F32, isOutput=False)
    cyct_d = nc.declare_dram_parameter("cyct", [b_per_core, C, s_total], BF16, isOutput=False)
    a3_d = nc.declare_dram_parameter("a3", [C, 3, C], BF16, isOutput=False)
    # rows: 0=e0, 1=e2 (edge bias corrections), 2=gbias, 3=sc_row(gamma*ln_w), 4=gb_row(gamma*ln_b)
    vec_d = nc.declare_dram_parameter("vecs", [6, C], F32, isOutput=False)
    onespad_d = nc.declare_dram_parameter("onespad", [C, 2 * tb_max], BF16, isOutput=False)
    out_d = nc.declare_dram_parameter("out", [b_per_core, s_total, n_total, C], BF16, isOutput=True)

    def n_batches():
        res = []
        n0 = 0
        while n0 < n_total:
            t = min(tb_max, n_total - n0)
            res.append((n0, t))
            n0 += t
        return res

    with tile.TileContext(nc) as tc, ExitStack() as ctx:
        singles = ctx.enter_context(tc.tile_pool(name="singles", bufs=1))
        xin = ctx.enter_context(tc.tile_pool(name="xin", bufs=6))
        ypool = ctx.enter_context(tc.tile_pool(name="ypool", bufs=tb_max + 4))
        y2pool = ctx.enter_context(tc.tile_pool(name="y2pool", bufs=3))
        cycp = ctx.enter_context(tc.tile_pool(name="cycp", bufs=2))
        ostage = ctx.enter_context(tc.tile_pool(name="ostage", bufs=2 * len(CH)))
        stats = ctx.enter_context(tc.tile_pool(name="stats", bufs=8))
        statsT = ctx.enter_context(tc.tile_pool(name="statsT", bufs=2 * len(CH) * 2 + 2))
        xtmp = ctx.enter_context(tc.tile_pool(name="xtmp", bufs=6))
        if any(m == "qfold" for m in chunk_modes):
            munqp = ctx.enter_context(tc.tile_pool(name="munqp", bufs=2))

        pxT = ctx.enter_context(tc.tile_pool(name="pxT", bufs=2, space="PSUM"))
        pz = ctx.enter_context(tc.tile_pool(name="pz", bufs=2, space="PSUM"))
        pstat = ctx.enter_context(tc.tile_pool(name="pstat", bufs=2, space="PSUM"))
        ptok = ctx.enter_context(tc.tile_pool(name="ptok", bufs=2, space="PSUM"))

        # --- constants ---
        identf = singles.tile([128, 128], F32)
        make_identity(nc, identf[:, :])
        identb = singles.tile([128, 128], BF16)
        nc.vector.tensor_copy(out=identb[:, :], in_=identf[:, :])

        a3_sb = singles.tile([C, 3, C], BF16)
        nc.sync.dma_start(out=a3_sb[:, :, :], in_=a3_d[:, :, :])

        vec_sb = singles.tile([6, C], F32)
        nc.sync.dma_start(out=vec_sb[:, :], in_=vec_d[:, :])
        gb_e0_col = singles.tile([C, 1], F32)
        nc.sync.dma_start(out=gb_e0_col[:, :], in_=vec_d[0:1, :].rearrange("a c -> c a"))
        gb_e2_col = singles.tile([C, 1], F32)
        nc.sync.dma_start(out=gb_e2_col[:, :], in_=vec_d[1:2, :].rearrange("a c -> c a"))
        gbias_col = singles.tile([C, 1], F32)
        nc.sync.dma_start(out=gbias_col[:, :], in_=vec_d[2:3, :].rearrange("a c -> c a"))

        ones1 = singles.tile([1, 1], F32)
        nc.vector.memset(ones1[:, :], 1.0)
        eps_col = singles.tile([128, 1], F32)
        nc.vector.memset(eps_col[:, :], LN_EPS)
        ones_rowb = singles.tile([1, C], F32)
        nc.vector.memset(ones_rowb[:, :], 1.0)
        F32R = mybir.dt.float32r

        ones_pad = singles.tile([C, 2 * tb_max], BF16)
        nc.sync.dma_start(out=ones_pad[:, :], in_=onespad_d[:, :])

        XDT = F32 if in_mode == "hwdge_f32" else BF16
        xident = identf if in_mode == "hwdge_f32" else identb

        if not fast_path:
            # per-channel gamma*ln_w / gamma*ln_b broadcast to all partitions
            wbc = singles.tile([128, C], F32)
            nc.sync.dma_start(out=wbc[0:1, :], in_=vec_d[3:4, :])
            nc.gpsimd.partition_broadcast(wbc[:, :], wbc[0:1, :])
            gbc = singles.tile([128, C], F32)
            nc.sync.dma_start(out=gbc[0:1, :], in_=vec_d[4:5, :])
            nc.gpsimd.partition_broadcast(gbc[:, :], gbc[0:1, :])

        # cb ring: 3 persistent padded buffers; edge columns zeroed once.
        cb_ring = []
        for r in range(3):
            cbt = singles.tile([C, SP], BF16, name=f"cbt{r}")
            nc.vector.memset(cbt[:, 0:2], 0.0)
            nc.vector.memset(cbt[:, SP - 2 : SP], 0.0)
            cb_ring.append(cbt)

        # --- engine warm-ups: touch DMA'd constants once ---
        pwarm = ptok.tile([128, C], F32, tag="ptok", name="pwarm")
        nc.tensor.matmul(out=pwarm[:, 0:128], lhsT=identf[:, :], rhs=identf[:, :],
                         is_transpose=True, start=True, stop=True)
        opw = min(128, 2 * tb_max)
        nc.tensor.matmul(out=pwarm[:, 0:opw], lhsT=a3_sb[:, 1, :], rhs=ones_pad[:, 0:opw],
                         start=True, stop=True)
        wscr = singles.tile([128, 1], F32)
        nc.scalar.activation(out=wscr[:, :], in_=gbias_col[:, :], func=AF.Square)
        nc.scalar.activation(out=wscr[:, :], in_=gb_e0_col[:, :], func=AF.Square)
        nc.scalar.activation(out=wscr[:, :], in_=gb_e2_col[:, :], func=AF.Square)

        rep_ctx = tc.For_i(0, repeat, 1) if repeat > 1 else None
        if rep_ctx is not None:
            ctx.enter_context(rep_ctx)

        pipeline = (pipe_mode > 0) and not ablate

        def chunk_mode(chi):
            if not fast_path:
                return "act"
            return chunk_modes[chi % len(chunk_modes)]

        need_qq = (not fast_path) or any(chunk_mode(c) in ("act", "xq") for c in range(len(CH)))
        need_mun = fast_path and any(chunk_mode(c) == "qfold" for c in range(len(CH)))

        def emit_stats_math_and_pq(st):
            tbn = st["tbn"]
            s1_ps, s2_ps = st["s1"], st["s2"]
            # mun = -mu (negated mean); msq = mun*mun == mu*mu
            mun = stats.tile([tb_max, s_total], F32, tag="stats", name="mun")
            nc.vector.tensor_scalar_mul(out=mun[0:tbn, :], in0=s1_ps[0:tbn, :], scalar1=-1.0 / C)
            var = stats.tile([tb_max, s_total], F32, tag="stats", name="var")
            nc.vector.tensor_scalar_mul(out=var[0:tbn, :], in0=s2_ps[0:tbn, :], scalar1=1.0 / C)
            msq = stats.tile([tb_max, s_total], F32, tag="stats", name="msq")
            nc.vector.tensor_tensor(out=msq[0:tbn, :], in0=mun[0:tbn, :], in1=mun[0:tbn, :], op=OP.mult)
            nc.vector.tensor_tensor(out=var[0:tbn, :], in0=var[0:tbn, :], in1=msq[0:tbn, :], op=OP.subtract)
            nc.scalar.activation(
                out=var[0:tbn, :], in_=var[0:tbn, :], func=AF.Sqrt,
                bias=eps_col[0:tbn, :], scale=1.0,
            )
            rstd = stats.tile([tb_max, s_total], F32, tag="stats", name="rstd")
            nc.vector.reciprocal(out=rstd[0:tbn, :], in_=var[0:tbn, :])
            pp = stats.tile([tb_max, s_total], F32, tag="stats", name="pp")
            nc.vector.tensor_scalar_mul(out=pp[0:tbn, :], in0=rstd[0:tbn, :], scalar1=float(gamma_sc))
            if need_qq:
                qq = stats.tile([tb_max, s_total], F32, tag="stats", name="qq")
                nc.vector.tensor_tensor(out=qq[0:tbn, :], in0=mun[0:tbn, :], in1=pp[0:tbn, :], op=OP.mult)
            if need_mun:
                munq = {}
                for chi, (s0, sc) in enumerate(CH):
                    if chunk_mode(chi) != "qfold":
                        continue
                    mq = munqp.tile([1, tb_max, sc], F32, tag=f"munq{chi}", name=f"munq{chi}")
                    nc.sync.dma_start(out=mq[0:1, 0:tbn, :], in_=mun[0:tbn, s0 : s0 + sc])
                    munq[chi] = mq
                st["munq"] = munq
            ppT = {}
            qqT = {}
            for chi, (s0, sc) in enumerate(CH):
                srcs = [("p", pp)]
                if chunk_mode(chi) in ("act", "xq"):
                    srcs.append(("q", qq))
                for name, srcm in srcs:
                    pt = ptok.tile([128, C], F32, tag="ptok", name="pt")
                    nc.tensor.matmul(
                        out=pt[0:sc, 0:tbn],
                        lhsT=srcm[0:tbn, s0 : s0 + sc],
                        rhs=identf[0:tbn, 0:tbn],
                        is_transpose=True,
                        start=True,
                        stop=True,
                    )
                    st_t = statsT.tile([128, tb_max], F32, tag="statsT", name="stt")
                    nc.vector.tensor_copy(out=st_t[0:sc, 0:tbn], in_=pt[0:sc, 0:tbn])
                    if name == "p":
                        ppT[chi] = st_t
                    else:
                        qqT[chi] = st_t
            st["ppT"], st["qqT"] = ppT, qqT

        def xslot(st, chi, j, sc):
            if "xh" in st:
                return st["xh"][chi][0:sc, st["xoff"] + j, :]
            return st["x"][(j // nb, chi)][0:sc, j % nb, :]

        def emit_B_tile(st):
            j = st["jB"]
            if j >= st["tbn"]:
                return
            st["jB"] = j + 1
            tbn, bb, n0 = st["tbn"], st["b"], st["n0"]
            y_tiles = st["y"]
            ppT, qqT = st["ppT"], st["qqT"]
            ot = st["ot"]
            nblk = j // nb
            if j % nb == 0:
                for chi in range(len(CH)):
                    ot[chi] = ostage.tile([128, nb, C], BF16, tag="ostage", name=f"ot{chi}")
                    nc.vector.memset(ot[chi][0:1, 0:1, 0:1], 0.0)
            for chi, (s0, sc) in enumerate(CH):
                mode = chunk_mode(chi)
                x_slot = xslot(st, chi, j, sc)
                if mode == "qfold":
                    # normal-MM transpose (yt stationary, identity moving) ->
                    # fp32 PSUM, then accumulate -mu broadcast over channels
                    bank = ptok.tile([128, C], F32, tag="ptok", name="bankf")
                    nc.tensor.matmul(
                        out=bank[0:sc, :],
                        lhsT=y_tiles[j][:, s0 : s0 + sc],
                        rhs=identb[0:128, 0:128],
                        start=True,
                        stop=False,
                    )
                    nc.tensor.matmul(
                        out=bank[0:sc, :],
                        lhsT=st["munq"][chi][0:1, j, :].bitcast(F32R),
                        rhs=ones_rowb[:, :].bitcast(F32R),
                        start=False,
                        stop=True,
                    )
                    nc.vector.scalar_tensor_tensor(
                        out=ot[chi][0:sc, j % nb, :],
                        in0=bank[0:sc, :],
                        scalar=ppT[chi][0:sc, j : j + 1],
                        in1=x_slot,
                        op0=OP.mult,
                        op1=OP.add,
                    )
                    continue
                bank = ptok.tile([128, C], BF16, tag="ptok", name="bank")
                nc.tensor.matmul(
                    out=bank[0:sc, :],
                    lhsT=y_tiles[j][:, s0 : s0 + sc],
                    rhs=identb[0:128, 0:128],
                    is_transpose=True,
                    start=True,
                    stop=True,
                )
                if fast_path and mode == "act":
                    tmp = xtmp.tile([128, C], BF16, tag="xtmp", name="tmp")
                    nc.scalar.activation(
                        out=tmp[0:sc, :], in_=bank[0:sc, :], func=AF.Identity,
                        bias=qqT[chi][0:sc, j : j + 1], scale=ppT[chi][0:sc, j : j + 1],
                    )
                    nc.vector.tensor_tensor(
                        out=ot[chi][0:sc, j % nb, :], in0=tmp[0:sc, :], in1=x_slot, op=OP.add,
                    )
                elif fast_path:
                    xq = xtmp.tile([128, C], BF16, tag="xtmp", name="xq")
                    if xq_engine == "pool":
                        nc.gpsimd.tensor_scalar_add(out=xq[0:sc, :], in0=x_slot,
                                                    scalar1=qqT[chi][0:sc, j : j + 1])
                    else:
                        nc.vector.tensor_scalar_add(out=xq[0:sc, :], in0=x_slot,
                                                    scalar1=qqT[chi][0:sc, j : j + 1])
                    nc.vector.scalar_tensor_tensor(
                        out=ot[chi][0:sc, j % nb, :],
                        in0=bank[0:sc, :],
                        scalar=ppT[chi][0:sc, j : j + 1],
                        in1=xq[0:sc, :],
                        op0=OP.mult,
                        op1=OP.add,
                    )
                else:
                    # general ln_w/ln_b path (correctness only; pp=gamma*rstd)
                    tmp = xtmp.tile([128, C], F32, tag="xtmpg", name="tmpg")
                    nc.scalar.activation(
                        out=tmp[0:sc, :], in_=bank[0:sc, :], func=AF.Identity,
                        bias=qqT[chi][0:sc, j : j + 1], scale=ppT[chi][0:sc, j : j + 1],
                    )
                    nc.vector.tensor_tensor(out=tmp[0:sc, :], in0=tmp[0:sc, :],
                                            in1=wbc[0:sc, :], op=OP.mult)
                    nc.vector.tensor_tensor(out=tmp[0:sc, :], in0=tmp[0:sc, :],
                                            in1=gbc[0:sc, :], op=OP.add)
                    nc.vector.tensor_tensor(
                        out=ot[chi][0:sc, j % nb, :], in0=tmp[0:sc, :], in1=x_slot, op=OP.add,
                    )
            if (j % nb == nb - 1) or (j == tbn - 1):
                nbw = (j % nb) + 1
                nst = n0 + nblk * nb
                for chi, (s0, sc) in enumerate(CH):
                    nc.sync.dma_start(
                        out=out_d[bb, s0 : s0 + sc, nst : nst + nbw, :],
                        in_=ot[chi][0:sc, 0:nbw, :],
                    )

        def drain_B(st):
            if st is None:
                return
            while st["jB"] < st["tbn"]:
                emit_B_tile(st)

        pending = None
        jglobal = 0

        big = in_mode == "swdge_big"
        if big:
            h1 = (n_total + 1) // 2
            halves = [(0, h1), (h1, n_total - h1)]
            half_seq = [(b_, h_) for b_ in range(b_per_core) for h_ in range(len(halves))]
            xhalf_tiles = {}

            def issue_half(idx):
                if idx >= len(half_seq) or half_seq[idx] in xhalf_tiles:
                    return
                b_, h_ = half_seq[idx]
                h0, hn = halves[h_]
                d = {}
                for chi, (s0, sc) in enumerate(CH):
                    xt = xin.tile([128, h1, C], BF16, tag="xin")
                    nc.gpsimd.dma_start(
                        out=xt[0:sc, 0:hn, :],
                        in_=x_d[b_, s0 : s0 + sc, h0 : h0 + hn, :],
                    )
                    d[chi] = xt
                xhalf_tiles[half_seq[idx]] = d

            issue_half(0)

            def batch_plan():
                plan = []
                for b_ in range(b_per_core):
                    for h_, (h0, hn) in enumerate(halves):
                        n0_ = h0
                        first = True
                        while n0_ < h0 + hn:
                            t = min(tb_max, h0 + hn - n0_)
                            plan.append((b_, h_, n0_, t, first))
                            first = False
                            n0_ += t
                return plan

            plan = batch_plan()
        else:
            plan = []
            for b_ in range(b_per_core):
                for (n0_, t) in n_batches():
                    plan.append((b_, None, n0_, t, False))

        last_b = None
        for (b, hcur, n0, tbn, half_first) in plan:
            if b != last_b:
                last_b = b
                cyc_sb = cycp.tile([C, s_total], BF16, tag="cycp")
                nc.sync.dma_start(out=cyc_sb[:, :], in_=cyct_d[b, :, :])
                cyc_touch = cycp.tile([128, 1], BF16, tag="cyct_touch")
                nc.vector.tensor_copy(out=cyc_touch[:, :], in_=cyc_sb[:, 0:1])
            if big and half_first:
                issue_half(half_seq.index((b, hcur)) + 1)
            if True:
                # ---------- PHASE A (with phase B of the previous batch interleaved) ----------
                s1_ps = pstat.tile([tb_max, s_total], F32, tag="pstat", name="s1_ps")
                s2_ps = pstat.tile([tb_max, s_total], F32, tag="pstat", name="s2_ps")
                st_cur = {"b": b, "n0": n0, "tbn": tbn, "x": {}, "y": {}, "jB": 0, "ot": {},
                          "s1": s1_ps, "s2": s2_ps}
                if big:
                    st_cur["xh"] = xhalf_tiles[(b, hcur)]
                    st_cur["xoff"] = n0 - halves[hcur][0]
                x_tiles, y_tiles = st_cur["x"], st_cur["y"]
                pend_stats = []

                def flush_stats(upto, tbn=tbn, s1_ps=s1_ps, s2_ps=s2_ps, pend_stats=pend_stats):
                    while pend_stats and pend_stats[0][0] <= upto:
                        jj, yt_, y2_ = pend_stats.pop(0)
                        win = ones_pad[:, tb_max - jj : 2 * tb_max - jj]
                        nc.tensor.matmul(
                            out=s1_ps[:, :], lhsT=win, rhs=yt_[:, :],
                            start=(jj == 0), stop=(jj == tbn - 1),
                        )
                        nc.tensor.matmul(
                            out=s2_ps[:, :], lhsT=win, rhs=y2_[:, :],
                            start=(jj == 0), stop=(jj == tbn - 1),
                        )

                def issue_block_loads(nblk_):
                    if big or nblk_ * nb >= tbn or (nblk_, 0) in x_tiles:
                        return
                    nbw = min(nb, tbn - nblk_ * nb)
                    for chi, (s0, sc) in enumerate(CH):
                        xt = xin.tile([128, nb, C], XDT, tag="xin")
                        xdma = nc.gpsimd if in_mode == "swdge_bf16" else nc.sync
                        xdma.dma_start(
                            out=xt[0:sc, 0:nbw, :],
                            in_=x_d[b, s0 : s0 + sc, n0 + nblk_ * nb : n0 + nblk_ * nb + nbw, :],
                        )
                        x_tiles[(nblk_, chi)] = xt

                issue_block_loads(0)
                for j in range(tbn):
                    nblk = j // nb
                    if j % nb == 0:
                        issue_block_loads(nblk + 1)

                    xT = pxT.tile([C, s_total], XDT, tag="pxT", name="xT")
                    for chi, (s0, sc) in enumerate(CH):
                        nc.tensor.matmul(
                            out=xT[:, s0 : s0 + sc],
                            lhsT=xslot(st_cur, chi, j, sc),
                            rhs=xident[0:sc, 0:sc],
                            is_transpose=True,
                            start=True,
                            stop=True,
                        )
                    cb = cb_ring[jglobal % 3]
                    jglobal += 1
                    nc.vector.tensor_tensor(
                        out=cb[:, 2 : 2 + s_total], in0=xT[:, :], in1=cyc_sb[:, :], op=OP.mult
                    )

                    z = pz.tile([C, s_total], F32, tag="pz", name="z")
                    if "noconv" in ablate:
                        nc.tensor.matmul(out=z[:, :], lhsT=a3_sb[:, 1, :],
                                         rhs=cb[:, 2 : 2 + s_total], start=True, stop=True)
                    else:
                        nc.tensor.matmul(out=z[:, :], lhsT=a3_sb[:, 1, :],
                                         rhs=cb[:, 2 : 2 + s_total], start=True, stop=False)
                        nc.tensor.matmul(out=z[:, :], lhsT=a3_sb[:, 0, :],
                                         rhs=cb[:, 1 : 1 + s_total], start=False, stop=False)
                        nc.tensor.matmul(out=z[:, :], lhsT=a3_sb[:, 2, :],
                                         rhs=cb[:, 3 : 3 + s_total], start=False, stop=True)

                    yt = ypool.tile([C, s_total], BF16, tag="ypool", name="yt")
                    nc.scalar.activation(
                        out=yt[:, :], in_=z[:, :], func=act_fn, bias=gbias_col[:, :], scale=1.0
                    )
                    nc.scalar.activation(
                        out=yt[:, 0:1], in_=z[:, 0:1], func=act_fn, bias=gb_e0_col[:, :], scale=1.0
                    )
                    nc.scalar.activation(
                        out=yt[:, s_total - 1 : s_total], in_=z[:, s_total - 1 : s_total],
                        func=act_fn, bias=gb_e2_col[:, :], scale=1.0
                    )
                    y_tiles[j] = yt
                    if "nostats" not in ablate:
                        y2 = y2pool.tile([C, s_total], BF16, tag="y2pool", name="y2")
                        if y2_engine == "pool":
                            nc.gpsimd.tensor_tensor(out=y2[:, :], in0=yt[:, :], in1=yt[:, :], op=OP.mult)
                        elif y2_engine == "act":
                            nc.scalar.activation(out=y2[:, :], in_=yt[:, :], func=AF.Square)
                        else:
                            nc.vector.tensor_tensor(out=y2[:, :], in0=yt[:, :], in1=yt[:, :], op=OP.mult)
                        pend_stats.append((j, yt, y2))
                        flush_stats(j - 2)
                    if pipeline and pending is not None and (pipe_mode == 1 or j % 2 == 0):
                        emit_B_tile(pending)

                if "nostats" not in ablate:
                    flush_stats(tbn)

                if "nophb" in ablate or "nostats" in ablate:
                    for j0 in range(0, tbn, nb):
                        nbw = min(nb, tbn - j0)
                        for chi, (s0, sc) in enumerate(CH):
                            otx = ostage.tile([128, nb, C], BF16, tag="ostage", name="otx")
                            nc.vector.tensor_copy(
                                out=otx[0:sc, 0:nbw, :],
                                in_=xslot(st_cur, chi, j0, sc).tensor_rearrange()
                                if False else x_tiles[(j0 // nb, chi)][0:sc, 0:nbw, :],
                            )
                            nc.sync.dma_start(
                                out=out_d[b, s0 : s0 + sc, n0 + j0 : n0 + j0 + nbw, :],
                                in_=otx[0:sc, 0:nbw, :],
                            )
                    continue

                drain_B(pending)
                emit_stats_math_and_pq(st_cur)
                if pipeline:
                    pending = st_cur
                else:
                    drain_B(st_cur)
                    pending = None

        drain_B(pending)
    nc.compile()
    return nc


# ------------------------- host side -------------------------

def _host_prep(inputs):
    seasonal = np.asarray(inputs["seasonal_component"], dtype=np.float32)
    cycle_index = np.asarray(inputs["cycle_index"])
    cycle_data = np.asarray(inputs["cycle_data"], dtype=np.float32)
    W_c = np.asarray(inputs["W_c"], dtype=np.float32)
    lin_b = np.asarray(inputs["lin_b"], dtype=np.float32)
    b_c = np.asarray(inputs["b_c"], dtype=np.float32)
    conv_w = np.asarray(inputs["conv_w"], dtype=np.float32)
    conv_b = np.asarray(inputs["conv_b"], dtype=np.float32)
    ln_w = np.asarray(inputs["ln_w"], dtype=np.float32)
    ln_b = np.asarray(inputs["ln_b"], dtype=np.float32)
    gamma = float(np.asarray(inputs["gamma"]))

    b_, s_, n_, c_ = seasonal.shape
    cl = cycle_data.shape[0]

    idx = (np.asarray(cycle_index)[:, None] % cl + np.arange(s_)[None, :]) % cl
    cyc = cycle_data[idx]  # [B,S,C]
    cycT = np.ascontiguousarray(cyc.transpose(0, 2, 1)).astype(ml_dtypes.bfloat16)

    w3 = conv_w[:, 0, :]  # [C,3]
    lb = lin_b + b_c
    a3 = np.ascontiguousarray(W_c.T[:, None, :] * w3.T[None, :, :]).astype(ml_dtypes.bfloat16)

    gbias = lb * (w3[:, 0] + w3[:, 1] + w3[:, 2]) + conv_b
    gb_e0 = gbias - lb * w3[:, 0]
    gb_e2 = gbias - lb * w3[:, 2]

    fast_path = bool(np.all(ln_w == ln_w[0]) and np.all(ln_b == 0.0))
    gamma_sc = gamma * float(ln_w[0]) if fast_path else gamma
    sc_row = ln_w.astype(np.float32)          # general path: per-channel ln_w
    gb_row = (gamma * ln_b).astype(np.float32)  # general path: gamma*ln_b

    u_edge = np.linalg.solve(W_c, -lb)
    vecs = np.stack([gb_e0, gb_e2, gbias, sc_row, gb_row, u_edge], axis=0).astype(np.float32)
    return seasonal, cycT, a3, vecs, fast_path, gamma_sc


def _make_onespad(tb_max=TB_MAX):
    op = np.zeros((C, 2 * tb_max), np.float32)
    op[:, tb_max] = 1.0
    return op.astype(ml_dtypes.bfloat16)


_prog_cache = {}


def kernel(**inputs) -> np.ndarray:
    seasonal, cycT, a3, vecs, fast_path, gamma_sc = _host_prep(inputs)
    b_, s_, n_, c_ = seasonal.shape
    assert c_ == C
    bpc = b_ // NCORES

    key = (bpc, n_, s_, fast_path, gamma_sc)
    if key not in _prog_cache:
        _prog_cache[key] = build_program(
            b_per_core=bpc, n_total=n_, s_total=s_,
            gamma_sc=gamma_sc, fast_path=fast_path,
        )
    nc = _prog_cache[key]

    in_maps = []
    for i in range(NCORES):
        in_maps.append(
            {
                "x": np.ascontiguousarray(seasonal[i * bpc : (i + 1) * bpc]),
                "cyct": np.ascontiguousarray(cycT[i * bpc : (i + 1) * bpc]),
                "a3": a3,
                "vecs": vecs,
                "onespad": _make_onespad(),
            }
        )
    res = run_bass_kernel_spmd(nc, in_maps, list(range(NCORES)))
    outs = [res.results[i]["out"] for i in range(NCORES)]
    return np.concatenate(outs, axis=0).astype(np.float32)


# revision 27
# speedup vs baseline: 1.0164x; 1.0164x over previous
"""Trainium2 Bass kernel for nn_EnhancedSeasonalModule (v2, bf16 compute).

Computation (reference):
  cyc[b,s,:]   = cycle_data[(cycle_index[b]+s) % CL]
  combined     = seasonal * cyc                              [B,S,N,C]
  transformed  = combined @ W_c^T + (lin_b + b_c)            (einsum bsnc,dc->bsnd)
  z            = depthwise_conv1d_k3_same(transformed, conv_w)  over s, per (b,n)
  y            = gelu_exact(z + conv_b)
  ln           = layernorm_C(y) * ln_w + ln_b
  out          = seasonal + gamma * ln

Strategy: data-parallel over batch (2 of 16 per core, 8 cores).
Per (b,n) tile [C=128 x S=288], channels on partitions; all bulk tensors are
bf16 (tolerance 2e-2 >> bf16 rounding):
  - x is DMA'd with an fp32->bf16 cast (SWDGE) in 6 large CONTIGUOUS
    per-(b, n-half, s-chunk) transfers (strided cast-DMAs cost ~2k Q7-built
    descriptors each and were ~130us slower); PE transposes bf16 [s,c]
    tiles into bf16 PSUM; DVE evacuates fused with the cyc multiply at
    2x_1P rate into a padded cb buffer whose edge columns stay zero,
    turning depthwise conv + linear into 3 PSUM-accumulated bf16 matmuls
    with A_k = diag(conv_w[:,k]) @ W_c; ACT applies exact GELU (interior
    linear bias folded; per-edge-column biases via 2 tiny ACTs); y^2 runs
    on the idle GPSIMD engine; LN channel sums come from shifted-ones
    matmuls accumulated into per-n-batch PSUM stats banks; the finish
    transposes y back to token layout (bf16 PSUM) and applies
    out = pp[s]*y^T + (x + qq[s]) with pp = gamma*rstd, qq = -mu*pp,
    split between DVE (tensor_scalar + scalar_tensor_tensor) and ACT
    (Identity with per-partition scale/bias + DVE add) to balance engines.
  - output is written bf16 and upcast on the host.
"""

import numpy as np
import ml_dtypes
from contextlib import ExitStack

import concourse.bass as bass
import concourse.bacc as bacc_mod
import concourse.tile as tile
from concourse import mybir
from concourse.bass_utils import run_bass_kernel_spmd
from concourse.masks import make_identity

F32 = mybir.dt.float32
BF16 = mybir.dt.bfloat16
AF = mybir.ActivationFunctionType
OP = mybir.AluOpType

B, S, N, C, CL = 16, 288, 170, 128, 24
LN_EPS = 1e-5
NCORES = 8
TB_MAX = 43


def _chunks(s_total):
    out = []
    s0 = 0
    while s0 < s_total:
        sc = min(128, s_total - s0)
        out.append((s0, sc))
        s0 += sc
    return out


def build_program(
    b_per_core: int,
    n_total: int,
    s_total: int,
    gamma_sc: float,
    tb_max: int = TB_MAX,
    nb: int = 16,
    use_f32r: bool = True,   # kept for signature compat; unused in v2
    fast_path: bool = True,
    act_fn=None,
    repeat: int = 1,
    ablate: str = "",
    pipe_mode: int = 1,
    chunk_modes=("xq", "act", "xq"),  # per-chunk LN-apply strategy
    y2_engine: str = "pool",  # pool | dve | act
    xq_engine: str = "dve",   # dve | pool
    in_mode: str = "swdge_big",  # swdge_big | swdge_bf16 | hwdge_f32
):
    nc = bacc_mod.Bacc("TRN2", target_bir_lowering=False)
    if act_fn is None:
        act_fn = AF.Gelu

    CH = _chunks(s_total)
    SP = s_total + 4  # cb layout: [pad, zero | combined (2..2+S) | zero, pad]

    x_d = nc.declare_dram_parameter("x", [b_per_core, s_total, n_total, C], F32, isOutput=False)
    cyct_d = nc.declare_dram_parameter("cyct", [b_per_core, C, s_total], BF16, isOutput=False)
    a3_d = nc.declare_dram_parameter("a3", [C, 3, C], BF16, isOutput=False)
    # rows: 0=e0, 1=e2 (edge bias corrections), 2=gbias, 3=sc_row(gamma*ln_w), 4=gb_row(gamma*ln_b)
    vec_d = nc.declare_dram_parameter("vecs", [6, C], F32, isOutput=False)
    onespad_d = nc.declare_dram_parameter("onespad", [C, 2 * tb_max], BF16, isOutput=False)
    out_d = nc.declare_dram_parameter("out", [b_per_core, s_total, n_total, C], BF16, isOutput=True)

    def n_batches():
        res = []
        n0 = 0
        while n0 < n_total:
            t = min(tb_max, n_total - n0)
            res.append((n0, t))
            n0 += t
        return res

    with tile.TileContext(nc) as tc, ExitStack() as ctx:
        singles = ctx.enter_context(tc.tile_pool(name="singles", bufs=1))
        xin = ctx.enter_context(tc.tile_pool(name="xin", bufs=6))
        ypool = ctx.enter_context(tc.tile_pool(name="ypool", bufs=tb_max + 4))
        y2pool = ctx.enter_context(tc.tile_pool(name="y2pool", bufs=3))
        cycp = ctx.enter_context(tc.tile_pool(name="cycp", bufs=2))
        ostage = ctx.enter_context(tc.tile_pool(name="ostage", bufs=2 * len(CH)))
        stats = ctx.enter_context(tc.tile_pool(name="stats", bufs=8))
        statsT = ctx.enter_context(tc.tile_pool(name="statsT", bufs=2 * len(CH) * 2 + 2))
        xtmp = ctx.enter_context(tc.tile_pool(name="xtmp", bufs=6))
        if any(m == "qfold" for m in chunk_modes):
            munqp = ctx.enter_context(tc.tile_pool(name="munqp", bufs=2))

        pxT = ctx.enter_context(tc.tile_pool(name="pxT", bufs=2, space="PSUM"))
        pz = ctx.enter_context(tc.tile_pool(name="pz", bufs=2, space="PSUM"))
        pstat = ctx.enter_context(tc.tile_pool(name="pstat", bufs=2, space="PSUM"))
        ptok = ctx.enter_context(tc.tile_pool(name="ptok", bufs=2, space="PSUM"))

        # --- constants ---
        identf = singles.tile([128, 128], F32)
        make_identity(nc, identf[:, :])
        identb = singles.tile([128, 128], BF16)
        nc.vector.tensor_copy(out=identb[:, :], in_=identf[:, :])

        a3_sb = singles.tile([C, 3, C], BF16)
        nc.sync.dma_start(out=a3_sb[:, :, :], in_=a3_d[:, :, :])

        vec_sb = singles.tile([6, C], F32)
        nc.sync.dma_start(out=vec_sb[:, :], in_=vec_d[:, :])
        gb_e0_col = singles.tile([C, 1], F32)
        nc.sync.dma_start(out=gb_e0_col[:, :], in_=vec_d[0:1, :].rearrange("a c -> c a"))
        gb_e2_col = singles.tile([C, 1], F32)
        nc.sync.dma_start(out=gb_e2_col[:, :], in_=vec_d[1:2, :].rearrange("a c -> c a"))
        gbias_col = singles.tile([C, 1], F32)
        nc.sync.dma_start(out=gbias_col[:, :], in_=vec_d[2:3, :].rearrange("a c -> c a"))

        ones1 = singles.tile([1, 1], F32)
        nc.vector.memset(ones1[:, :], 1.0)
        eps_col = singles.tile([128, 1], F32)
        nc.vector.memset(eps_col[:, :], LN_EPS)
        ones_rowb = singles.tile([1, C], F32)
        nc.vector.memset(ones_rowb[:, :], 1.0)
        F32R = mybir.dt.float32r

        ed2 = singles.tile([2, C], BF16)
        nc.vector.tensor_copy(out=ed2[:, :], in_=vec_sb[0:2, :])
        sel2 = singles.tile([2, 2], BF16)
        nc.vector.tensor_copy(out=sel2[:, :], in_=identf[0:2, 0:2])

        ones_pad = singles.tile([C, 2 * tb_max], BF16)
        nc.sync.dma_start(out=ones_pad[:, :], in_=onespad_d[:, :])

        XDT = F32 if in_mode == "hwdge_f32" else BF16
        xident = identf if in_mode == "hwdge_f32" else identb

        if not fast_path:
            # per-channel gamma*ln_w / gamma*ln_b broadcast to all partitions
            wbc = singles.tile([128, C], F32)
            nc.sync.dma_start(out=wbc[0:1, :], in_=vec_d[3:4, :])
            nc.gpsimd.partition_broadcast(wbc[:, :], wbc[0:1, :])
            gbc = singles.tile([128, C], F32)
            nc.sync.dma_start(out=gbc[0:1, :], in_=vec_d[4:5, :])
            nc.gpsimd.partition_broadcast(gbc[:, :], gbc[0:1, :])

        # cb ring: 3 persistent padded buffers; edge columns zeroed once.
        cb_ring = []
        for r in range(3):
            cbt = singles.tile([C, SP], BF16, name=f"cbt{r}")
            nc.vector.memset(cbt[:, 0:2], 0.0)
            nc.vector.memset(cbt[:, SP - 2 : SP], 0.0)
            cb_ring.append(cbt)

        # --- engine warm-ups: touch DMA'd constants once ---
        pwarm = ptok.tile([128, C], F32, tag="ptok", name="pwarm")
        nc.tensor.matmul(out=pwarm[:, 0:128], lhsT=identf[:, :], rhs=identf[:, :],
                         is_transpose=True, start=True, stop=True)
        opw = min(128, 2 * tb_max)
        nc.tensor.matmul(out=pwarm[:, 0:opw], lhsT=a3_sb[:, 1, :], rhs=ones_pad[:, 0:opw],
                         start=True, stop=True)
        wscr = singles.tile([128, 1], F32)
        nc.scalar.activation(out=wscr[:, :], in_=gbias_col[:, :], func=AF.Square)
        nc.scalar.activation(out=wscr[:, :], in_=gb_e0_col[:, :], func=AF.Square)
        nc.scalar.activation(out=wscr[:, :], in_=gb_e2_col[:, :], func=AF.Square)

        rep_ctx = tc.For_i(0, repeat, 1) if repeat > 1 else None
        if rep_ctx is not None:
            ctx.enter_context(rep_ctx)

        pipeline = (pipe_mode > 0) and not ablate

        def chunk_mode(chi):
            if not fast_path:
                return "act"
            return chunk_modes[chi % len(chunk_modes)]

        need_qq = (not fast_path) or any(chunk_mode(c) in ("act", "xq") for c in range(len(CH)))
        need_mun = fast_path and any(chunk_mode(c) == "qfold" for c in range(len(CH)))

        def emit_stats_math_and_pq(st):
            tbn = st["tbn"]
            s1_ps, s2_ps = st["s1"], st["s2"]
            # mun = -mu (negated mean); msq = mun*mun == mu*mu
            mun = stats.tile([tb_max, s_total], F32, tag="stats", name="mun")
            nc.vector.tensor_scalar_mul(out=mun[0:tbn, :], in0=s1_ps[0:tbn, :], scalar1=-1.0 / C)
            var = stats.tile([tb_max, s_total], F32, tag="stats", name="var")
            nc.vector.tensor_scalar_mul(out=var[0:tbn, :], in0=s2_ps[0:tbn, :], scalar1=1.0 / C)
            msq = stats.tile([tb_max, s_total], F32, tag="stats", name="msq")
            nc.vector.tensor_tensor(out=msq[0:tbn, :], in0=mun[0:tbn, :], in1=mun[0:tbn, :], op=OP.mult)
            nc.vector.tensor_tensor(out=var[0:tbn, :], in0=var[0:tbn, :], in1=msq[0:tbn, :], op=OP.subtract)
            nc.scalar.activation(
                out=var[0:tbn, :], in_=var[0:tbn, :], func=AF.Sqrt,
                bias=eps_col[0:tbn, :], scale=1.0,
            )
            rstd = stats.tile([tb_max, s_total], F32, tag="stats", name="rstd")
            nc.vector.reciprocal(out=rstd[0:tbn, :], in_=var[0:tbn, :])
            pp = stats.tile([tb_max, s_total], F32, tag="stats", name="pp")
            nc.vector.tensor_scalar_mul(out=pp[0:tbn, :], in0=rstd[0:tbn, :], scalar1=float(gamma_sc))
            if need_qq:
                qq = stats.tile([tb_max, s_total], F32, tag="stats", name="qq")
                nc.vector.tensor_tensor(out=qq[0:tbn, :], in0=mun[0:tbn, :], in1=pp[0:tbn, :], op=OP.mult)
            if need_mun:
                munq = {}
                for chi, (s0, sc) in enumerate(CH):
                    if chunk_mode(chi) != "qfold":
                        continue
                    mq = munqp.tile([1, tb_max, sc], F32, tag=f"munq{chi}", name=f"munq{chi}")
                    nc.sync.dma_start(out=mq[0:1, 0:tbn, :], in_=mun[0:tbn, s0 : s0 + sc])
                    munq[chi] = mq
                st["munq"] = munq
            ppT = {}
            qqT = {}
            for chi, (s0, sc) in enumerate(CH):
                srcs = [("p", pp)]
                if chunk_mode(chi) in ("act", "xq"):
                    srcs.append(("q", qq))
                for name, srcm in srcs:
                    pt = ptok.tile([128, C], F32, tag="ptok", name="pt")
                    nc.tensor.matmul(
                        out=pt[0:sc, 0:tbn],
                        lhsT=srcm[0:tbn, s0 : s0 + sc],
                        rhs=identf[0:tbn, 0:tbn],
                        is_transpose=True,
                        start=True,
                        stop=True,
                    )
                    st_t = statsT.tile([128, tb_max], F32, tag="statsT", name="stt")
                    nc.vector.tensor_copy(out=st_t[0:sc, 0:tbn], in_=pt[0:sc, 0:tbn])
                    if name == "p":
                        ppT[chi] = st_t
                    else:
                        qqT[chi] = st_t
            st["ppT"], st["qqT"] = ppT, qqT

        def xslot(st, chi, j, sc):
            if "xh" in st:
                return st["xh"][chi][0:sc, st["xoff"] + j, :]
            return st["x"][(j // nb, chi)][0:sc, j % nb, :]

        def emit_B_tile(st):
            j = st["jB"]
            if j >= st["tbn"]:
                return
            st["jB"] = j + 1
            tbn, bb, n0 = st["tbn"], st["b"], st["n0"]
            y_tiles = st["y"]
            ppT, qqT = st["ppT"], st["qqT"]
            ot = st["ot"]
            nblk = j // nb
            if j % nb == 0:
                for chi in range(len(CH)):
                    ot[chi] = ostage.tile([128, nb, C], BF16, tag="ostage", name=f"ot{chi}")
                    nc.vector.memset(ot[chi][0:1, 0:1, 0:1], 0.0)
            for chi, (s0, sc) in enumerate(CH):
                mode = chunk_mode(chi)
                x_slot = xslot(st, chi, j, sc)
                if mode == "qfold":
                    # normal-MM transpose (yt stationary, identity moving) ->
                    # fp32 PSUM, then accumulate -mu broadcast over channels
                    bank = ptok.tile([128, C], F32, tag="ptok", name="bankf")
                    nc.tensor.matmul(
                        out=bank[0:sc, :],
                        lhsT=y_tiles[j][:, s0 : s0 + sc],
                        rhs=identb[0:128, 0:128],
                        start=True,
                        stop=False,
                    )
                    nc.tensor.matmul(
                        out=bank[0:sc, :],
                        lhsT=st["munq"][chi][0:1, j, :].bitcast(F32R),
                        rhs=ones_rowb[:, :].bitcast(F32R),
                        start=False,
                        stop=True,
                    )
                    nc.vector.scalar_tensor_tensor(
                        out=ot[chi][0:sc, j % nb, :],
                        in0=bank[0:sc, :],
                        scalar=ppT[chi][0:sc, j : j + 1],
                        in1=x_slot,
                        op0=OP.mult,
                        op1=OP.add,
                    )
                    continue
                bank = ptok.tile([128, C], BF16, tag="ptok", name="bank")
                nc.tensor.matmul(
                    out=bank[0:sc, :],
                    lhsT=y_tiles[j][:, s0 : s0 + sc],
                    rhs=identb[0:128, 0:128],
                    is_transpose=True,
                    start=True,
                    stop=True,
                )
                if fast_path and mode == "act":
                    tmp = xtmp.tile([128, C], BF16, tag="xtmp", name="tmp")
                    nc.scalar.activation(
                        out=tmp[0:sc, :], in_=bank[0:sc, :], func=AF.Identity,
                        bias=qqT[chi][0:sc, j : j + 1], scale=ppT[chi][0:sc, j : j + 1],
                    )
                    nc.vector.tensor_tensor(
                        out=ot[chi][0:sc, j % nb, :], in0=tmp[0:sc, :], in1=x_slot, op=OP.add,
                    )
                elif fast_path:
                    xq = xtmp.tile([128, C], BF16, tag="xtmp", name="xq")
                    if xq_engine == "pool":
                        nc.gpsimd.tensor_scalar_add(out=xq[0:sc, :], in0=x_slot,
                                                    scalar1=qqT[chi][0:sc, j : j + 1])
                    else:
                        nc.vector.tensor_scalar_add(out=xq[0:sc, :], in0=x_slot,
                                                    scalar1=qqT[chi][0:sc, j : j + 1])
                    nc.vector.scalar_tensor_tensor(
                        out=ot[chi][0:sc, j % nb, :],
                        in0=bank[0:sc, :],
                        scalar=ppT[chi][0:sc, j : j + 1],
                        in1=xq[0:sc, :],
                        op0=OP.mult,
                        op1=OP.add,
                    )
                else:
                    # general ln_w/ln_b path (correctness only; pp=gamma*rstd)
                    tmp = xtmp.tile([128, C], F32, tag="xtmpg", name="tmpg")
                    nc.scalar.activation(
                        out=tmp[0:sc, :], in_=bank[0:sc, :], func=AF.Identity,
                        bias=qqT[chi][0:sc, j : j + 1], scale=ppT[chi][0:sc, j : j + 1],
                    )
                    nc.vector.tensor_tensor(out=tmp[0:sc, :], in0=tmp[0:sc, :],
                                            in1=wbc[0:sc, :], op=OP.mult)
                    nc.vector.tensor_tensor(out=tmp[0:sc, :], in0=tmp[0:sc, :],
                                            in1=gbc[0:sc, :], op=OP.add)
                    nc.vector.tensor_tensor(
                        out=ot[chi][0:sc, j % nb, :], in0=tmp[0:sc, :], in1=x_slot, op=OP.add,
                    )
            if (j % nb == nb - 1) or (j == tbn - 1):
                nbw = (j % nb) + 1
                nst = n0 + nblk * nb
                for chi, (s0, sc) in enumerate(CH):
                    nc.sync.dma_start(
                        out=out_d[bb, s0 : s0 + sc, nst : nst + nbw, :],
                        in_=ot[chi][0:sc, 0:nbw, :],
                    )

        def drain_B(st):
            if st is None:
                return
            while st["jB"] < st["tbn"]:
                emit_B_tile(st)

        pending = None
        jglobal = 0

        big = in_mode == "swdge_big"
        if big:
            h1 = (n_total + 1) // 2
            halves = [(0, h1), (h1, n_total - h1)]
            half_seq = [(b_, h_) for b_ in range(b_per_core) for h_ in range(len(halves))]
            xhalf_tiles = {}

            def issue_half(idx):
                if idx >= len(half_seq) or half_seq[idx] in xhalf_tiles:
                    return
                b_, h_ = half_seq[idx]
                h0, hn = halves[h_]
                d = {}
                for chi, (s0, sc) in enumerate(CH):
                    xt = xin.tile([128, h1, C], BF16, tag="xin")
                    nc.gpsimd.dma_start(
                        out=xt[0:sc, 0:hn, :],
                        in_=x_d[b_, s0 : s0 + sc, h0 : h0 + hn, :],
                    )
                    d[chi] = xt
                xhalf_tiles[half_seq[idx]] = d

            issue_half(0)

            def batch_plan():
                plan = []
                for b_ in range(b_per_core):
                    for h_, (h0, hn) in enumerate(halves):
                        n0_ = h0
                        first = True
                        while n0_ < h0 + hn:
                            t = min(tb_max, h0 + hn - n0_)
                            plan.append((b_, h_, n0_, t, first))
                            first = False
                            n0_ += t
                return plan

            plan = batch_plan()
        else:
            plan = []
            for b_ in range(b_per_core):
                for (n0_, t) in n_batches():
                    plan.append((b_, None, n0_, t, False))

        last_b = None
        for (b, hcur, n0, tbn, half_first) in plan:
            if b != last_b:
                last_b = b
                cyc_sb = cycp.tile([C, s_total], BF16, tag="cycp")
                nc.sync.dma_start(out=cyc_sb[:, :], in_=cyct_d[b, :, :])
                cyc_touch = cycp.tile([128, 1], BF16, tag="cyct_touch")
                nc.vector.tensor_copy(out=cyc_touch[:, :], in_=cyc_sb[:, 0:1])
            if big and half_first:
                issue_half(half_seq.index((b, hcur)) + 1)
            if True:
                # ---------- PHASE A (with phase B of the previous batch interleaved) ----------
                s1_ps = pstat.tile([tb_max, s_total], F32, tag="pstat", name="s1_ps")
                s2_ps = pstat.tile([tb_max, s_total], F32, tag="pstat", name="s2_ps")
                st_cur = {"b": b, "n0": n0, "tbn": tbn, "x": {}, "y": {}, "jB": 0, "ot": {},
                          "s1": s1_ps, "s2": s2_ps}
                if big:
                    st_cur["xh"] = xhalf_tiles[(b, hcur)]
                    st_cur["xoff"] = n0 - halves[hcur][0]
                x_tiles, y_tiles = st_cur["x"], st_cur["y"]
                pend_stats = []

                def flush_stats(upto, tbn=tbn, s1_ps=s1_ps, s2_ps=s2_ps, pend_stats=pend_stats):
                    while pend_stats and pend_stats[0][0] <= upto:
                        jj, yt_, y2_ = pend_stats.pop(0)
                        win = ones_pad[:, tb_max - jj : 2 * tb_max - jj]
                        nc.tensor.matmul(
                            out=s1_ps[:, :], lhsT=win, rhs=yt_[:, :],
                            start=(jj == 0), stop=(jj == tbn - 1),
                        )
                        nc.tensor.matmul(
                            out=s2_ps[:, :], lhsT=win, rhs=y2_[:, :],
                            start=(jj == 0), stop=(jj == tbn - 1),
                        )

                def issue_block_loads(nblk_):
                    if big or nblk_ * nb >= tbn or (nblk_, 0) in x_tiles:
                        return
                    nbw = min(nb, tbn - nblk_ * nb)
                    for chi, (s0, sc) in enumerate(CH):
                        xt = xin.tile([128, nb, C], XDT, tag="xin")
                        xdma = nc.gpsimd if in_mode == "swdge_bf16" else nc.sync
                        xdma.dma_start(
                            out=xt[0:sc, 0:nbw, :],
                            in_=x_d[b, s0 : s0 + sc, n0 + nblk_ * nb : n0 + nblk_ * nb + nbw, :],
                        )
                        x_tiles[(nblk_, chi)] = xt

                issue_block_loads(0)
                for j in range(tbn):
                    nblk = j // nb
                    if j % nb == 0:
                        issue_block_loads(nblk + 1)

                    xT = pxT.tile([C, s_total], XDT, tag="pxT", name="xT")
                    for chi, (s0, sc) in enumerate(CH):
                        nc.tensor.matmul(
                            out=xT[:, s0 : s0 + sc],
                            lhsT=xslot(st_cur, chi, j, sc),
                            rhs=xident[0:sc, 0:sc],
                            is_transpose=True,
                            start=True,
                            stop=True,
                        )
                    cb = cb_ring[jglobal % 3]
                    jglobal += 1
                    nc.vector.tensor_tensor(
                        out=cb[:, 2 : 2 + s_total], in0=xT[:, :], in1=cyc_sb[:, :], op=OP.mult
                    )

                    z = pz.tile([C, s_total], F32, tag="pz", name="z")
                    if "noconv" in ablate:
                        nc.tensor.matmul(out=z[:, :], lhsT=a3_sb[:, 1, :],
                                         rhs=cb[:, 2 : 2 + s_total], start=True, stop=True)
                    else:
                        nc.tensor.matmul(out=z[:, :], lhsT=a3_sb[:, 1, :],
                                         rhs=cb[:, 2 : 2 + s_total], start=True, stop=False)
                        nc.tensor.matmul(out=z[:, :], lhsT=a3_sb[:, 0, :],
                                         rhs=cb[:, 1 : 1 + s_total], start=False, stop=False)
                        nc.tensor.matmul(out=z[:, :], lhsT=a3_sb[:, 2, :],
                                         rhs=cb[:, 3 : 3 + s_total], start=False, stop=False)
                        zedge = bass.AP(tensor=z.tensor, offset=z.offset,
                                        ap=[list(z.ap[0]), [s_total - 1, 2]])
                        nc.tensor.matmul(out=zedge, lhsT=ed2[:, :], rhs=sel2[:, :],
                                         start=False, stop=True)

                    yt = ypool.tile([C, s_total], BF16, tag="ypool", name="yt")
                    nc.scalar.activation(
                        out=yt[:, :], in_=z[:, :], func=act_fn, bias=gbias_col[:, :], scale=1.0
                    )
                    y_tiles[j] = yt
                    if "nostats" not in ablate:
                        y2 = y2pool.tile([C, s_total], BF16, tag="y2pool", name="y2")
                        if y2_engine == "pool":
                            nc.gpsimd.tensor_tensor(out=y2[:, :], in0=yt[:, :], in1=yt[:, :], op=OP.mult)
                        elif y2_engine == "act":
                            nc.scalar.activation(out=y2[:, :], in_=yt[:, :], func=AF.Square)
                        else:
                            nc.vector.tensor_tensor(out=y2[:, :], in0=yt[:, :], in1=yt[:, :], op=OP.mult)
                        pend_stats.append((j, yt, y2))
                        flush_stats(j - 2)
                    if pipeline and pending is not None and (pipe_mode == 1 or j % 2 == 0):
                        emit_B_tile(pending)

                if "nostats" not in ablate:
                    flush_stats(tbn)

                if "nophb" in ablate or "nostats" in ablate:
                    for j0 in range(0, tbn, nb):
                        nbw = min(nb, tbn - j0)
                        for chi, (s0, sc) in enumerate(CH):
                            otx = ostage.tile([128, nb, C], BF16, tag="ostage", name="otx")
                            nc.vector.tensor_copy(
                                out=otx[0:sc, 0:nbw, :],
                                in_=xslot(st_cur, chi, j0, sc).tensor_rearrange()
                                if False else x_tiles[(j0 // nb, chi)][0:sc, 0:nbw, :],
                            )
                            nc.sync.dma_start(
                                out=out_d[b, s0 : s0 + sc, n0 + j0 : n0 + j0 + nbw, :],
                                in_=otx[0:sc, 0:nbw, :],
                            )
                    continue

                drain_B(pending)
                emit_stats_math_and_pq(st_cur)
                if pipeline:
                    pending = st_cur
                else:
                    drain_B(st_cur)
                    pending = None

        drain_B(pending)
    nc.compile()
    return nc


# ------------------------- host side -------------------------

def _host_prep(inputs):
    seasonal = np.asarray(inputs["seasonal_component"], dtype=np.float32)
    cycle_index = np.asarray(inputs["cycle_index"])
    cycle_data = np.asarray(inputs["cycle_data"], dtype=np.float32)
    W_c = np.asarray(inputs["W_c"], dtype=np.float32)
    lin_b = np.asarray(inputs["lin_b"], dtype=np.float32)
    b_c = np.asarray(inputs["b_c"], dtype=np.float32)
    conv_w = np.asarray(inputs["conv_w"], dtype=np.float32)
    conv_b = np.asarray(inputs["conv_b"], dtype=np.float32)
    ln_w = np.asarray(inputs["ln_w"], dtype=np.float32)
    ln_b = np.asarray(inputs["ln_b"], dtype=np.float32)
    gamma = float(np.asarray(inputs["gamma"]))

    b_, s_, n_, c_ = seasonal.shape
    cl = cycle_data.shape[0]

    idx = (np.asarray(cycle_index)[:, None] % cl + np.arange(s_)[None, :]) % cl
    cyc = cycle_data[idx]  # [B,S,C]
    cycT = np.ascontiguousarray(cyc.transpose(0, 2, 1)).astype(ml_dtypes.bfloat16)

    w3 = conv_w[:, 0, :]  # [C,3]
    lb = lin_b + b_c
    a3 = np.ascontiguousarray(W_c.T[:, None, :] * w3.T[None, :, :]).astype(ml_dtypes.bfloat16)

    gbias = lb * (w3[:, 0] + w3[:, 1] + w3[:, 2]) + conv_b
    gb_e0 = -lb * w3[:, 0]   # edge bias DELTA at s=0
    gb_e2 = -lb * w3[:, 2]   # edge bias DELTA at s=S-1

    fast_path = bool(np.all(ln_w == ln_w[0]) and np.all(ln_b == 0.0))
    gamma_sc = gamma * float(ln_w[0]) if fast_path else gamma
    sc_row = ln_w.astype(np.float32)          # general path: per-channel ln_w
    gb_row = (gamma * ln_b).astype(np.float32)  # general path: gamma*ln_b

    u_edge = np.linalg.solve(W_c, -lb)
    vecs = np.stack([gb_e0, gb_e2, gbias, sc_row, gb_row, u_edge], axis=0).astype(np.float32)
    return seasonal, cycT, a3, vecs, fast_path, gamma_sc


def _make_onespad(tb_max=TB_MAX):
    op = np.zeros((C, 2 * tb_max), np.float32)
    op[:, tb_max] = 1.0
    return op.astype(ml_dtypes.bfloat16)


_prog_cache = {}


def kernel(**inputs) -> np.ndarray:
    seasonal, cycT, a3, vecs, fast_path, gamma_sc = _host_prep(inputs)
    b_, s_, n_, c_ = seasonal.shape
    assert c_ == C
    bpc = b_ // NCORES

    key = (bpc, n_, s_, fast_path, gamma_sc)
    if key not in _prog_cache:
        _prog_cache[key] = build_program(
            b_per_core=bpc, n_total=n_, s_total=s_,
            gamma_sc=gamma_sc, fast_path=fast_path,
        )
    nc = _prog_cache[key]

    in_maps = []
    for i in range(NCORES):
        in_maps.append(
            {
                "x": np.ascontiguousarray(seasonal[i * bpc : (i + 1) * bpc]),
                "cyct": np.ascontiguousarray(cycT[i * bpc : (i + 1) * bpc]),
                "a3": a3,
                "vecs": vecs,
                "onespad": _make_onespad(),
            }
        )
    res = run_bass_kernel_spmd(nc, in_maps, list(range(NCORES)))
    outs = [res.results[i]["out"] for i in range(NCORES)]
    return np.concatenate(outs, axis=0).astype(np.float32)
